# revision 12
# baseline (speedup 1.0000x reference)
"""Trainium2 Bass kernel for the DOC contrastive-loss module (epoch>=1 path).

Strategy (data-parallel over T, one frame per NeuronCore, 8 cores):

The reference computes, per frame, an L2-normalized pixel-feature Gram matrix
sim = f^T f / TEMP over N=H*W pixels, then
    pos    = exp(diag(sim))            (== exp(1/TEMP), since features are unit vectors)
    neg_n  = sum_{m in bg} exp(sim[n,m])
    l_n    = log(pos + neg_n + eps) - log(pos)
    frame_loss = mean_{n in fg} l_n,   loss = mean over valid frames.
Only the fg-rows x bg-cols block of the Gram matrix is ever needed, and pos is
a constant.  So the host (sharding step) partitions each frame's pixels into
fg/bg sets from the label maps, and each core computes:
    G = ffg_raw^T @ fbg_normalized     (PE, bf16, K=C=256)
    E = exp(G * rnorm_fg[row] / TEMP)  (ACT, fused row-sum accumulate -> neg)
    frame_loss from neg                (ACT/DVE epilogue, partition-sum via PE)
Zero-padding of the gathered fg/bg pixel sets is corrected exactly on device:
a padded bg column contributes exp(0)=1 per row (subtracted via the A constant)
and padded fg rows are masked out of the final sum.

Host-side work is limited to label selection / fg-bg index computation (the
sharding decision) and the trivial 8-way mean over frames.
"""

import functools
import math

import ml_dtypes
import numpy as np

import concourse.bass as bass
import concourse.mybir as mybir
import concourse.tile as tile
from concourse import bacc
from concourse.bass_utils import run_bass_kernel_spmd

TEMP = 0.07
EPS = 1e-8
THRESH = 0.0
LOGPOS = 1.0 / TEMP  # log(pos) where pos = exp(1/TEMP)
POS = math.exp(LOGPOS)

AF = mybir.ActivationFunctionType
ALU = mybir.AluOpType
AX = mybir.AxisListType

_NC_CACHE: dict = {}
LAST_RESULTS = None  # BassKernelResults of the most recent run (for profiling)


# ---------------------------------------------------------------------------
# Force every activation into the one table set that covers {Exp, Ln, Copy,
# Identity} so the program loads activation tables exactly once instead of
# ping-ponging between the exp and ln sets (~1.3us per load, serialized).
_ONE_SET = "natural_log_exp_and_others"
_orig_get_tables = None


def _patched_get_tables(arch):
    tabs = _orig_get_tables(arch)
    return {
        name: (funcs if name == _ONE_SET else frozenset())
        for name, funcs in tabs.items()
    }


def _install_act_table_patch():
    global _orig_get_tables
    if _orig_get_tables is not None:
        return
    from concourse import hw_specs

    _orig_get_tables = hw_specs.get_activation_tables
    patched = functools.cache(_patched_get_tables)
    hw_specs.get_activation_tables = patched
    bacc.get_activation_tables = patched


def _bg_chunks(BGP: int):
    """Chunk the bg axis into 1024-wide pieces; fold any remainder into the
    last chunk (so it is 1024..1536 wide -> at most 3 PSUM banks)."""
    n = max(1, BGP // 1024)
    chunks = [(i * 1024, 1024) for i in range(n)]
    rem = BGP - n * 1024
    off, w = chunks[-1]
    chunks[-1] = (off, w + rem)
    assert chunks[-1][1] <= 1536
    return chunks


def _build_nc(FG_TILES: int, BG_TILES: int, CB: int):
    """SPMD single-core program: fg-rows x bg-cols masked softmax-denominator."""
    _install_act_table_patch()
    f32 = mybir.dt.float32
    bf16 = mybir.dt.bfloat16
    FGP = 128 * FG_TILES
    BGP = 128 * BG_TILES

    nc = bacc.Bacc("TRN2", target_bir_lowering=False, debug=False)

    ffg_d = nc.dram_tensor("ffg", [CB, 128, FGP], bf16, kind="ExternalInput")
    fbg_d = nc.dram_tensor("fbg", [CB, 128, BGP], bf16, kind="ExternalInput")
    rm_d = nc.dram_tensor("rowmask", [128, FG_TILES], f32, kind="ExternalInput")
    cst_d = nc.dram_tensor("consts", [128, 2], f32, kind="ExternalInput")
    out_d = nc.dram_tensor("out", [1, 1], f32, kind="ExternalOutput")

    chunks = _bg_chunks(BGP)
    NB = len(chunks)
    LW = chunks[-1][1]  # last (widest) chunk

    with tile.TileContext(nc) as tc:
        with (
            tc.tile_pool(name="persist", bufs=1) as P,
            tc.tile_pool(name="scratch", bufs=3) as S,
            tc.tile_pool(name="sp_psum", bufs=1, space="PSUM") as SP,
            tc.tile_pool(name="mm_psum", bufs=2, space="PSUM") as MP,
            tc.tile_pool(name="ml_psum", bufs=1, space="PSUM") as ML,
        ):
            # ---- input DMA first, issue spread across idle engine sequencers
            # (each 2D DMA instruction costs ~650ns of sequencer issue time) ----
            HF = FGP // 2
            c0w = chunks[0][1]
            fbgb = [P.tile([128, BGP], bf16, name=f"fbgb_{c}") for c in range(CB)]
            ffgb = [P.tile([128, FGP], bf16, name=f"ffgb_{c}") for c in range(CB)]
            dma_engines = [nc.sync, nc.scalar, nc.gpsimd]
            di = 0

            def next_eng():
                nonlocal di
                e = dma_engines[di % len(dma_engines)]
                di += 1
                return e

            for c in range(CB):
                next_eng().dma_start(fbgb[c][:, 0:c0w], fbg_d[c, :, 0:c0w])
            for c in range(CB):
                next_eng().dma_start(ffgb[c][:, 0:HF], ffg_d[c, :, 0:HF])
                next_eng().dma_start(ffgb[c][:, HF:FGP], ffg_d[c, :, HF:FGP])
            for off, w in chunks[1:]:
                for c in range(CB):
                    next_eng().dma_start(fbgb[c][:, off : off + w], fbg_d[c, :, off : off + w])
            consts = P.tile([128, 2], f32)
            nc.gpsimd.dma_start(consts[:], cst_d[:, :])
            rmask = P.tile([128, FG_TILES], f32)
            nc.gpsimd.dma_start(rmask[:], rm_d[:, :])

            ones_bf = P.tile([128, 128], bf16)
            nc.vector.memset(ones_bf[:], 1.0)
            ones_f = P.tile([128, 1], f32)
            nc.vector.memset(ones_f[:], 1.0)
            bias_tiny = P.tile([128, 1], f32)
            nc.vector.memset(bias_tiny[:], 1e-30)
            bias_lnt = P.tile([128, 1], f32)
            nc.vector.memset(bias_lnt[:], float(math.log(1.0 / TEMP)))

            # ---- PE warmup: ~2.5us of dummy matmuls while DMA lands, so the
            # HAM clock-gate reaches 2.4GHz before the real matmuls start ----
            wu = MP.tile([128, 1024], f32, name="g0", tag="g0")
            for _ in range(24):
                nc.tensor.matmul(wu[:, 0:128], ones_bf[:, :], ones_bf[:, :])

            # ---- bg pipeline, chunk-major: f2 -> ones-matmul -> ln -> exp -> mul
            f2bg = [P.tile([128, BGP], bf16, name=f"f2bg_{c}") for c in range(CB)]
            lnbg = P.tile([128, BGP], f32)
            rnbg = P.tile([128, BGP], bf16)
            fbgn = [P.tile([128, BGP], bf16, name=f"fbgn_{c}") for c in range(CB)]

            def bg_norm_chunk(off, w, tag):
                for c in range(CB):
                    nc.vector.tensor_mul(
                        f2bg[c][:, off : off + w],
                        fbgb[c][:, off : off + w],
                        fbgb[c][:, off : off + w],
                    )
                if tag == "g0":
                    ps = MP.tile([128, w], f32, name="g0", tag="g0")
                else:
                    ps = ML.tile([128, w], f32, name="gl", tag="gl")
                for c in range(CB):
                    for s in range(0, w, 512):
                        ws = min(512, w - s)
                        nc.tensor.matmul(
                            ps[:, s : s + ws],
                            ones_bf[:, :],
                            f2bg[c][:, off + s : off + s + ws],
                            start=(c == 0),
                            stop=(c == CB - 1),
                        )
                # ln(norm2); +1e-30 keeps padded (all-zero) columns finite
                nc.scalar.activation(
                    lnbg[:, off : off + w], ps[:, :], AF.Ln, bias=bias_tiny[:, :]
                )
                # rnorm = exp(-0.5*ln(norm2)) = 1/sqrt(norm2); padded cols stay 0
                nc.scalar.activation(
                    rnbg[:, off : off + w], lnbg[:, off : off + w], AF.Exp, scale=-0.5
                )
                for c in range(CB):
                    nc.vector.tensor_mul(
                        fbgn[c][:, off : off + w],
                        fbgb[c][:, off : off + w],
                        rnbg[:, off : off + w],
                    )

            bg_norm_chunk(*chunks[0], tag="g0")

            # ---- fg norms -> per-row ACT scale 1/(norm*TEMP) ----
            f2fg = [P.tile([128, FGP], bf16, name=f"f2fg_{c}") for c in range(CB)]
            for j0 in (0, HF):
                for c in range(CB):
                    nc.vector.tensor_mul(
                        f2fg[c][:, j0 : j0 + HF],
                        ffgb[c][:, j0 : j0 + HF],
                        ffgb[c][:, j0 : j0 + HF],
                    )
            ps2 = SP.tile([128, FG_TILES], f32, name="sp", tag="sp")
            for i in range(FG_TILES):
                for c in range(CB):
                    nc.tensor.matmul(
                        ps2[:, i : i + 1],
                        f2fg[c][:, 128 * i : 128 * (i + 1)],
                        ones_bf[:, 0:1],
                        start=(c == 0),
                        stop=(c == CB - 1),
                    )
            lnfg = P.tile([128, FG_TILES], f32)
            nc.scalar.activation(lnfg[:, :], ps2[:, :], AF.Ln, bias=bias_tiny[:, :])
            scfg = P.tile([128, FG_TILES], f32)
            # scale_fg = exp(-0.5*ln(norm2) + ln(1/TEMP)) = 1/(norm*TEMP)
            nc.scalar.activation(
                scfg[:, :], lnfg[:, :], AF.Exp, scale=-0.5, bias=bias_lnt[:, :]
            )

            for off, w in chunks[1:]:
                bg_norm_chunk(off, w, tag="gl")

            # ---- main loop: G tiles -> exp with fused row-sum on ACT ----
            negacc = P.tile([128, FG_TILES * NB], f32)
            for mi in range(FG_TILES):
                gts = []
                for j, (off, w) in enumerate(chunks):
                    if j < NB - 1:
                        gt = MP.tile([128, w], f32, name="g0", tag="g0")
                    else:
                        gt = ML.tile([128, w], f32, name="gl", tag="gl")
                    gts.append(gt)
                for c in range(CB):
                    lhsT = ffgb[c][:, 128 * mi : 128 * (mi + 1)]
                    for gt, (off, w) in zip(gts, chunks):
                        for s in range(0, w, 512):
                            ws = min(512, w - s)
                            nc.tensor.matmul(
                                gt[:, s : s + ws],
                                lhsT,
                                fbgn[c][:, off + s : off + s + ws],
                                start=(c == 0),
                                stop=(c == CB - 1),
                            )
                for j, (gt, (off, w)) in enumerate(zip(gts, chunks)):
                    es = S.tile([128, LW], bf16, name="es", tag=f"es{j}")
                    nc.scalar.activation(
                        es[:, 0:w],
                        gt[:, :],
                        AF.Exp,
                        scale=scfg[:, mi : mi + 1],
                        accum_out=negacc[:, mi * NB + j : mi * NB + j + 1],
                    )

            # ---- epilogue ----
            negsum = P.tile([128, FG_TILES], f32)
            nc.vector.tensor_reduce(
                negsum[:, :],
                negacc[:, :].rearrange("p (m j) -> p m j", j=NB),
                axis=AX.X,
                op=ALU.add,
            )
            plog = P.tile([128, FG_TILES], f32)
            # A = POS + EPS - n_bg_pad folds the padded-column correction into the bias
            nc.scalar.activation(plog[:, :], negsum[:, :], AF.Ln, bias=consts[:, 0:1])
            masked = P.tile([128, FG_TILES], f32)
            nc.vector.scalar_tensor_tensor(
                masked[:, :], plog[:, :], -LOGPOS, rmask[:, :], op0=ALU.add, op1=ALU.mult
            )
            red = P.tile([128, 1], f32)
            nc.vector.tensor_reduce(red[:, :], masked[:, :], axis=AX.X, op=ALU.add)
            ps3 = SP.tile([1, 1], f32, name="sp", tag="sp")
            nc.tensor.matmul(ps3[:, :], red[:, :], ones_f[:, :])
            res = P.tile([1, 1], f32)
            nc.scalar.activation(res[:, :], ps3[:, :], AF.Copy, scale=consts[0:1, 1:2])
            nc.sync.dma_start(out_d[:, :], res[:, :])

    nc.compile()
    return nc


def _get_nc(FG_TILES: int, BG_TILES: int, CB: int):
    key = (FG_TILES, BG_TILES, CB)
    if key not in _NC_CACHE:
        _NC_CACHE[key] = _build_nc(FG_TILES, BG_TILES, CB)
    return _NC_CACHE[key]


def kernel(**inputs) -> np.ndarray:
    cur = np.asarray(inputs["current_preds"], dtype=np.float32)
    hist = np.asarray(inputs["history_preds"], dtype=np.float32)
    feats = np.asarray(inputs["features"], dtype=np.float32)

    T = cur.shape[0]
    C = feats.shape[1]
    N = int(np.prod(cur.shape[1:]))
    CB = C // 128

    # ---- labels (fp32 math mirroring the reference) ----
    cb = (cur > 0.5).astype(np.float32).reshape(T, -1)
    hb = (hist > 0.5).astype(np.float32).reshape(T, -1)
    e1 = (cb * hb).sum(axis=1, dtype=np.float32)
    e2 = cb.sum(axis=1, dtype=np.float32) + hb.sum(axis=1, dtype=np.float32)
    m1 = (np.float32(2.0) * e1 + np.float32(EPS)) / (e2 + np.float32(EPS))
    m2 = (e1 + np.float32(EPS)) / (e2 - e1 + np.float32(EPS))
    dev = np.float32(1.0) - (m1 + m2) / np.float32(2.0)
    use_curr = dev <= np.float32(THRESH)
    labels = np.where(use_curr[:, None, None, None], cur, hist).astype(np.float32)

    lbl = labels.reshape(T, N)
    fg = lbl > 0.5
    nfg = fg.sum(axis=1).astype(np.int64)
    nbg = N - nfg
    valid = (nfg > 0) & (nbg > 0)

    FG_TILES = max(1, int(-(-int(nfg.max()) // 128)))
    BG_TILES = max(1, int(-(-int(nbg.max()) // 128)))
    # last merged bg chunk must stay within 3 PSUM banks (<=1536 wide)
    while 128 * BG_TILES >= 1024 and (128 * BG_TILES) % 1024 > 512:
        BG_TILES += 1
    FGP, BGP = 128 * FG_TILES, 128 * BG_TILES

    in_maps = []
    for t in range(T):
        f = feats[t].reshape(C, N)
        m = fg[t]
        ffg = np.zeros((C, FGP), dtype=ml_dtypes.bfloat16)
        ffg[:, : nfg[t]] = f[:, m].astype(ml_dtypes.bfloat16)
        fbg = np.zeros((C, BGP), dtype=ml_dtypes.bfloat16)
        fbg[:, : nbg[t]] = f[:, ~m].astype(ml_dtypes.bfloat16)
        rowmask = (
            (np.arange(FGP).reshape(FG_TILES, 128).T < nfg[t]).astype(np.float32)
        )
        A = np.float32(POS + EPS - float(BGP - nbg[t]))
        inv_nfg = np.float32(1.0 / max(float(nfg[t]), 1.0))
        consts = np.tile(np.array([[A, inv_nfg]], dtype=np.float32), (128, 1))
        in_maps.append(
            {
                "ffg": np.ascontiguousarray(ffg.reshape(CB, 128, FGP)),
                "fbg": np.ascontiguousarray(fbg.reshape(CB, 128, BGP)),
                "rowmask": np.ascontiguousarray(rowmask),
                "consts": consts,
            }
        )

    nc = _get_nc(FG_TILES, BG_TILES, CB)
    res = run_bass_kernel_spmd(nc, in_maps, core_ids=list(range(T)))
    global LAST_RESULTS
    LAST_RESULTS = res

    fls = np.array([res.results[t]["out"][0, 0] for t in range(T)], dtype=np.float32)
    n_valid = int(valid.sum())
    if n_valid > 0:
        loss = np.float32((fls * valid.astype(np.float32)).sum() / max(n_valid, 1))
    else:
        loss = np.float32(0.0)
    return labels, np.asarray(loss, dtype=np.float32)


# revision 15
# speedup vs baseline: 1.0038x; 1.0038x over previous
"""Trainium2 Bass kernel for the DOC contrastive-loss module (epoch>=1 path).

Strategy (data-parallel over T, one frame per NeuronCore, 8 cores):

The reference computes, per frame, an L2-normalized pixel-feature Gram matrix
sim = f^T f / TEMP over N=H*W pixels, then
    pos    = exp(diag(sim))            (== exp(1/TEMP), since features are unit vectors)
    neg_n  = sum_{m in bg} exp(sim[n,m])
    l_n    = log(pos + neg_n + eps) - log(pos)
    frame_loss = mean_{n in fg} l_n,   loss = mean over valid frames.
Only the fg-rows x bg-cols block of the Gram matrix is ever needed, and pos is
a constant.  So the host (sharding step) partitions each frame's pixels into
fg/bg sets from the label maps, and each core computes:
    G = ffg_raw^T @ fbg_normalized     (PE, bf16, K=C=256)
    E = exp(G * rnorm_fg[row] / TEMP)  (ACT, fused row-sum accumulate -> neg)
    frame_loss from neg                (ACT/DVE epilogue, partition-sum via PE)
Zero-padding of the gathered fg/bg pixel sets is corrected exactly on device:
a padded bg column contributes exp(0)=1 per row (subtracted via the A constant)
and padded fg rows are masked out of the final sum.

Host-side work is limited to label selection / fg-bg index computation (the
sharding decision) and the trivial 8-way mean over frames.
"""

import functools
import math

import ml_dtypes
import numpy as np

import concourse.bass as bass
import concourse.mybir as mybir
import concourse.tile as tile
from concourse import bacc
from concourse.bass_utils import run_bass_kernel_spmd

TEMP = 0.07
EPS = 1e-8
THRESH = 0.0
LOGPOS = 1.0 / TEMP  # log(pos) where pos = exp(1/TEMP)
POS = math.exp(LOGPOS)

AF = mybir.ActivationFunctionType
ALU = mybir.AluOpType
AX = mybir.AxisListType

_NC_CACHE: dict = {}
LAST_RESULTS = None  # BassKernelResults of the most recent run (for profiling)


# ---------------------------------------------------------------------------
# Force every activation into the one table set that covers {Exp, Ln, Copy,
# Identity} so the program loads activation tables exactly once instead of
# ping-ponging between the exp and ln sets (~1.3us per load, serialized).
_ONE_SET = "natural_log_exp_and_others"
_orig_get_tables = None


def _patched_get_tables(arch):
    tabs = _orig_get_tables(arch)
    return {
        name: (funcs if name == _ONE_SET else frozenset())
        for name, funcs in tabs.items()
    }


def _install_act_table_patch():
    global _orig_get_tables
    if _orig_get_tables is not None:
        return
    from concourse import hw_specs

    _orig_get_tables = hw_specs.get_activation_tables
    patched = functools.cache(_patched_get_tables)
    hw_specs.get_activation_tables = patched
    bacc.get_activation_tables = patched


def _bg_chunks(BGP: int):
    """Chunk the bg axis into 1024-wide pieces; fold any remainder into the
    last chunk (so it is 1024..1536 wide -> at most 3 PSUM banks)."""
    n = max(1, BGP // 1024)
    chunks = [(i * 1024, 1024) for i in range(n)]
    rem = BGP - n * 1024
    off, w = chunks[-1]
    chunks[-1] = (off, w + rem)
    assert chunks[-1][1] <= 1536
    return chunks


def _build_nc(FG_TILES: int, BG_TILES: int, CB: int):
    """SPMD single-core program: fg-rows x bg-cols masked softmax-denominator."""
    _install_act_table_patch()
    f32 = mybir.dt.float32
    bf16 = mybir.dt.bfloat16
    FGP = 128 * FG_TILES
    BGP = 128 * BG_TILES

    nc = bacc.Bacc("TRN2", target_bir_lowering=False, debug=False)

    ffg_d = nc.dram_tensor("ffg", [CB, 128, FGP], bf16, kind="ExternalInput")
    fbg_d = nc.dram_tensor("fbg", [CB, 128, BGP], bf16, kind="ExternalInput")
    rm_d = nc.dram_tensor("rowmask", [128, FG_TILES], f32, kind="ExternalInput")
    cst_d = nc.dram_tensor("consts", [128, 2], f32, kind="ExternalInput")
    out_d = nc.dram_tensor("out", [1, 1], f32, kind="ExternalOutput")

    chunks = _bg_chunks(BGP)
    NB = len(chunks)
    LW = chunks[-1][1]  # last (widest) chunk

    with tile.TileContext(nc) as tc:
        with (
            tc.tile_pool(name="persist", bufs=1) as P,
            tc.tile_pool(name="scratch", bufs=3) as S,
            tc.tile_pool(name="sp_psum", bufs=1, space="PSUM") as SP,
            tc.tile_pool(name="mm_psum", bufs=2, space="PSUM") as MP,
            tc.tile_pool(name="ml_psum", bufs=1, space="PSUM") as ML,
        ):
            # ---- input DMA: single engine, dependency order (HW queue rings are
            # FIFO, so first-issued descriptors complete first: bg chunk0, then
            # fg halves, then the bg tail) ----
            HF = FGP // 2
            c0w = chunks[0][1]
            fbgb = [P.tile([128, BGP], bf16, name=f"fbgb_{c}") for c in range(CB)]
            ffgb = [P.tile([128, FGP], bf16, name=f"ffgb_{c}") for c in range(CB)]
            for c in range(CB):
                nc.sync.dma_start(fbgb[c][:, 0:c0w], fbg_d[c, :, 0:c0w])
            for j0 in (0, HF):
                for c in range(CB):
                    nc.sync.dma_start(
                        ffgb[c][:, j0 : j0 + HF], ffg_d[c, :, j0 : j0 + HF]
                    )
            for off, w in chunks[1:]:
                for c in range(CB):
                    nc.sync.dma_start(fbgb[c][:, off : off + w], fbg_d[c, :, off : off + w])
            consts = P.tile([128, 2], f32)
            nc.gpsimd.dma_start(consts[:], cst_d[:, :])
            rmask = P.tile([128, FG_TILES], f32)
            nc.gpsimd.dma_start(rmask[:], rm_d[:, :])

            ones_bf = P.tile([128, 128], bf16)
            nc.vector.memset(ones_bf[:], 1.0)
            ones_f = P.tile([128, 1], f32)
            nc.vector.memset(ones_f[:], 1.0)
            bias_tiny = P.tile([128, 1], f32)
            nc.vector.memset(bias_tiny[:], 1e-30)
            bias_lnt = P.tile([128, 1], f32)
            nc.vector.memset(bias_lnt[:], float(math.log(1.0 / TEMP)))

            # ---- PE warmup: ~2.5us of dummy matmuls while DMA lands, so the
            # HAM clock-gate reaches 2.4GHz before the real matmuls start ----
            wu = MP.tile([128, 1024], f32, name="g0", tag="g0")
            for _ in range(24):
                nc.tensor.matmul(wu[:, 0:128], ones_bf[:, :], ones_bf[:, :])

            # ---- bg pipeline, chunk-major: f2 -> ones-matmul -> ln -> exp -> mul
            f2bg = [P.tile([128, BGP], bf16, name=f"f2bg_{c}") for c in range(CB)]
            lnbg = P.tile([128, BGP], f32)
            rnbg = P.tile([128, BGP], bf16)
            fbgn = [P.tile([128, BGP], bf16, name=f"fbgn_{c}") for c in range(CB)]

            def bg_norm_chunk(off, w, tag, sub=1024):
                """sub < w pipelines the chain at finer granularity (chunk0)."""
                for s0 in range(0, w, sub):
                    sw = min(sub, w - s0)
                    for c in range(CB):
                        nc.vector.tensor_mul(
                            f2bg[c][:, off + s0 : off + s0 + sw],
                            fbgb[c][:, off + s0 : off + s0 + sw],
                            fbgb[c][:, off + s0 : off + s0 + sw],
                        )
                if tag == "g0":
                    ps = MP.tile([128, w], f32, name="g0", tag="g0")
                else:
                    ps = ML.tile([128, w], f32, name="gl", tag="gl")
                for s in range(0, w, 512):
                    ws = min(512, w - s)
                    for c in range(CB):
                        nc.tensor.matmul(
                            ps[:, s : s + ws],
                            ones_bf[:, :],
                            f2bg[c][:, off + s : off + s + ws],
                            start=(c == 0),
                            stop=(c == CB - 1),
                        )
                for s0 in range(0, w, sub):
                    sw = min(sub, w - s0)
                    # ln(norm2); +1e-30 keeps padded (all-zero) columns finite
                    nc.scalar.activation(
                        lnbg[:, off + s0 : off + s0 + sw],
                        ps[:, s0 : s0 + sw],
                        AF.Ln,
                        bias=bias_tiny[:, :],
                    )
                    # rnorm = exp(-0.5*ln) = 1/sqrt(norm2); padded cols stay 0
                    nc.scalar.activation(
                        rnbg[:, off + s0 : off + s0 + sw],
                        lnbg[:, off + s0 : off + s0 + sw],
                        AF.Exp,
                        scale=-0.5,
                    )
                    for c in range(CB):
                        nc.vector.tensor_mul(
                            fbgn[c][:, off + s0 : off + s0 + sw],
                            fbgb[c][:, off + s0 : off + s0 + sw],
                            rnbg[:, off + s0 : off + s0 + sw],
                        )

            bg_norm_chunk(*chunks[0], tag="g0", sub=512)

            # ---- fg norms -> per-row ACT scale 1/(norm*TEMP) ----
            # squares split between GpSimd (c0, otherwise idle) and Vector (c1)
            f2fg = [P.tile([128, FGP], bf16, name=f"f2fg_{c}") for c in range(CB)]
            for j0 in (0, HF):
                for c in range(CB):
                    eng = nc.gpsimd if c == 0 else nc.vector
                    eng.tensor_mul(
                        f2fg[c][:, j0 : j0 + HF],
                        ffgb[c][:, j0 : j0 + HF],
                        ffgb[c][:, j0 : j0 + HF],
                    )
            ps2 = SP.tile([128, FG_TILES], f32, name="sp", tag="sp")
            for i in range(FG_TILES):
                for c in range(CB):
                    nc.tensor.matmul(
                        ps2[:, i : i + 1],
                        f2fg[c][:, 128 * i : 128 * (i + 1)],
                        ones_bf[:, 0:1],
                        start=(c == 0),
                        stop=(c == CB - 1),
                    )
            lnfg = P.tile([128, FG_TILES], f32)
            nc.scalar.activation(lnfg[:, :], ps2[:, :], AF.Ln, bias=bias_tiny[:, :])
            scfg = P.tile([128, FG_TILES], f32)
            # scale_fg = exp(-0.5*ln(norm2) + ln(1/TEMP)) = 1/(norm*TEMP)
            nc.scalar.activation(
                scfg[:, :], lnfg[:, :], AF.Exp, scale=-0.5, bias=bias_lnt[:, :]
            )

            for off, w in chunks[1:]:
                bg_norm_chunk(off, w, tag="gl")

            # ---- main loop: G tiles -> exp with fused row-sum on ACT ----
            # The wide "gl" chunk is single-buffered (PSUM bank budget); that
            # hides fine mid-loop but would serialize PE behind ACT at the very
            # end, so the last row-tile instead uses double-buffered g0 slots
            # plus a small SP-bank remainder.
            last_chunks = []
            off = 0
            while off + 1024 <= BGP:
                last_chunks.append((off, 1024, "g0"))
                off += 1024
            if off < BGP:
                last_chunks.append((off, BGP - off, "sp"))
            NBMAX = max(NB, len(last_chunks))
            negacc = P.tile([128, FG_TILES * NBMAX], f32)
            nc.vector.memset(negacc[:], 0.0)
            for mi in range(FG_TILES):
                if mi == FG_TILES - 1:
                    mychunks = last_chunks
                else:
                    mychunks = [
                        (off, w, "g0" if j < NB - 1 else "gl")
                        for j, (off, w) in enumerate(chunks)
                    ]
                gts = []
                for off, w, tag in mychunks:
                    if tag == "g0":
                        gt = MP.tile([128, w], f32, name="g0", tag="g0")
                    elif tag == "gl":
                        gt = ML.tile([128, w], f32, name="gl", tag="gl")
                    else:
                        gt = SP.tile([128, w], f32, name="sp", tag="sp")
                    gts.append(gt)
                for c in range(CB):
                    lhsT = ffgb[c][:, 128 * mi : 128 * (mi + 1)]
                    for gt, (off, w, tag) in zip(gts, mychunks):
                        for s in range(0, w, 512):
                            ws = min(512, w - s)
                            nc.tensor.matmul(
                                gt[:, s : s + ws],
                                lhsT,
                                fbgn[c][:, off + s : off + s + ws],
                                start=(c == 0),
                                stop=(c == CB - 1),
                            )
                for j, (gt, (off, w, tag)) in enumerate(zip(gts, mychunks)):
                    es = S.tile([128, LW], bf16, name="es", tag=f"es{j}")
                    nc.scalar.activation(
                        es[:, 0:w],
                        gt[:, :],
                        AF.Exp,
                        scale=scfg[:, mi : mi + 1],
                        accum_out=negacc[:, mi * NBMAX + j : mi * NBMAX + j + 1],
                    )

            # ---- epilogue ----
            negsum = P.tile([128, FG_TILES], f32)
            nc.vector.tensor_reduce(
                negsum[:, :],
                negacc[:, :].rearrange("p (m j) -> p m j", j=NBMAX),
                axis=AX.X,
                op=ALU.add,
            )
            plog = P.tile([128, FG_TILES], f32)
            # A = POS + EPS - n_bg_pad folds the padded-column correction into the bias
            nc.scalar.activation(plog[:, :], negsum[:, :], AF.Ln, bias=consts[:, 0:1])
            masked = P.tile([128, FG_TILES], f32)
            nc.vector.scalar_tensor_tensor(
                masked[:, :], plog[:, :], -LOGPOS, rmask[:, :], op0=ALU.add, op1=ALU.mult
            )
            red = P.tile([128, 1], f32)
            nc.vector.tensor_reduce(red[:, :], masked[:, :], axis=AX.X, op=ALU.add)
            ps3 = SP.tile([1, 1], f32, name="sp", tag="sp")
            nc.tensor.matmul(ps3[:, :], red[:, :], ones_f[:, :])
            res = P.tile([1, 1], f32)
            nc.scalar.activation(res[:, :], ps3[:, :], AF.Copy, scale=consts[0:1, 1:2])
            nc.sync.dma_start(out_d[:, :], res[:, :])

    nc.compile()
    return nc


def _get_nc(FG_TILES: int, BG_TILES: int, CB: int):
    key = (FG_TILES, BG_TILES, CB)
    if key not in _NC_CACHE:
        _NC_CACHE[key] = _build_nc(FG_TILES, BG_TILES, CB)
    return _NC_CACHE[key]


def kernel(**inputs) -> np.ndarray:
    cur = np.asarray(inputs["current_preds"], dtype=np.float32)
    hist = np.asarray(inputs["history_preds"], dtype=np.float32)
    feats = np.asarray(inputs["features"], dtype=np.float32)

    T = cur.shape[0]
    C = feats.shape[1]
    N = int(np.prod(cur.shape[1:]))
    CB = C // 128

    # ---- labels (fp32 math mirroring the reference) ----
    cb = (cur > 0.5).astype(np.float32).reshape(T, -1)
    hb = (hist > 0.5).astype(np.float32).reshape(T, -1)
    e1 = (cb * hb).sum(axis=1, dtype=np.float32)
    e2 = cb.sum(axis=1, dtype=np.float32) + hb.sum(axis=1, dtype=np.float32)
    m1 = (np.float32(2.0) * e1 + np.float32(EPS)) / (e2 + np.float32(EPS))
    m2 = (e1 + np.float32(EPS)) / (e2 - e1 + np.float32(EPS))
    dev = np.float32(1.0) - (m1 + m2) / np.float32(2.0)
    use_curr = dev <= np.float32(THRESH)
    labels = np.where(use_curr[:, None, None, None], cur, hist).astype(np.float32)

    lbl = labels.reshape(T, N)
    fg = lbl > 0.5
    nfg = fg.sum(axis=1).astype(np.int64)
    nbg = N - nfg
    valid = (nfg > 0) & (nbg > 0)

    FG_TILES = max(1, int(-(-int(nfg.max()) // 128)))
    BG_TILES = max(1, int(-(-int(nbg.max()) // 128)))
    # last merged bg chunk must stay within 3 PSUM banks (<=1536 wide)
    while 128 * BG_TILES >= 1024 and (128 * BG_TILES) % 1024 > 512:
        BG_TILES += 1
    FGP, BGP = 128 * FG_TILES, 128 * BG_TILES

    in_maps = []
    for t in range(T):
        f = feats[t].reshape(C, N)
        m = fg[t]
        ffg = np.zeros((C, FGP), dtype=ml_dtypes.bfloat16)
        ffg[:, : nfg[t]] = f[:, m].astype(ml_dtypes.bfloat16)
        fbg = np.zeros((C, BGP), dtype=ml_dtypes.bfloat16)
        fbg[:, : nbg[t]] = f[:, ~m].astype(ml_dtypes.bfloat16)
        rowmask = (
            (np.arange(FGP).reshape(FG_TILES, 128).T < nfg[t]).astype(np.float32)
        )
        A = np.float32(POS + EPS - float(BGP - nbg[t]))
        inv_nfg = np.float32(1.0 / max(float(nfg[t]), 1.0))
        consts = np.tile(np.array([[A, inv_nfg]], dtype=np.float32), (128, 1))
        in_maps.append(
            {
                "ffg": np.ascontiguousarray(ffg.reshape(CB, 128, FGP)),
                "fbg": np.ascontiguousarray(fbg.reshape(CB, 128, BGP)),
                "rowmask": np.ascontiguousarray(rowmask),
                "consts": consts,
            }
        )

    nc = _get_nc(FG_TILES, BG_TILES, CB)
    res = run_bass_kernel_spmd(nc, in_maps, core_ids=list(range(T)))
    global LAST_RESULTS
    LAST_RESULTS = res

    fls = np.array([res.results[t]["out"][0, 0] for t in range(T)], dtype=np.float32)
    n_valid = int(valid.sum())
    if n_valid > 0:
        loss = np.float32((fls * valid.astype(np.float32)).sum() / max(n_valid, 1))
    else:
        loss = np.float32(0.0)
    return labels, np.asarray(loss, dtype=np.float32)


# revision 17
# speedup vs baseline: 1.0514x; 1.0475x over previous
"""Trainium2 Bass kernel for the DOC contrastive-loss module (epoch>=1 path).

Strategy (data-parallel over T, one frame per NeuronCore, 8 cores):

The reference computes, per frame, an L2-normalized pixel-feature Gram matrix
sim = f^T f / TEMP over N=H*W pixels, then
    pos    = exp(diag(sim))            (== exp(1/TEMP), since features are unit vectors)
    neg_n  = sum_{m in bg} exp(sim[n,m])
    l_n    = log(pos + neg_n + eps) - log(pos)
    frame_loss = mean_{n in fg} l_n,   loss = mean over valid frames.
Only the fg-rows x bg-cols block of the Gram matrix is ever needed, and pos is
a constant.  So the host (sharding step) partitions each frame's pixels into
fg/bg sets from the label maps, and each core computes:
    G = ffg_raw^T @ fbg_normalized     (PE, bf16, K=C=256)
    E = exp(G * rnorm_fg[row] / TEMP)  (ACT, fused row-sum accumulate -> neg)
    frame_loss from neg                (ACT/DVE epilogue, partition-sum via PE)
Zero-padding of the gathered fg/bg pixel sets is corrected exactly on device:
a padded bg column contributes exp(0)=1 per row (subtracted via the A constant)
and padded fg rows are masked out of the final sum.

Host-side work is limited to label selection / fg-bg index computation (the
sharding decision) and the trivial 8-way mean over frames.
"""

import functools
import math

import ml_dtypes
import numpy as np

import concourse.bass as bass
import concourse.mybir as mybir
import concourse.tile as tile
from concourse import bacc
from concourse.bass_utils import run_bass_kernel_spmd

TEMP = 0.07
EPS = 1e-8
THRESH = 0.0
LOGPOS = 1.0 / TEMP  # log(pos) where pos = exp(1/TEMP)
POS = math.exp(LOGPOS)

AF = mybir.ActivationFunctionType
ALU = mybir.AluOpType
AX = mybir.AxisListType

_NC_CACHE: dict = {}
LAST_RESULTS = None  # BassKernelResults of the most recent run (for profiling)


# ---------------------------------------------------------------------------
# Force every activation into the one table set that covers {Exp, Ln, Copy,
# Identity} so the program loads activation tables exactly once instead of
# ping-ponging between the exp and ln sets (~1.3us per load, serialized).
_ONE_SET = "natural_log_exp_and_others"
_orig_get_tables = None


def _patched_get_tables(arch):
    tabs = _orig_get_tables(arch)
    return {
        name: (funcs if name == _ONE_SET else frozenset())
        for name, funcs in tabs.items()
    }


def _install_act_table_patch():
    global _orig_get_tables
    if _orig_get_tables is not None:
        return
    from concourse import hw_specs

    _orig_get_tables = hw_specs.get_activation_tables
    patched = functools.cache(_patched_get_tables)
    hw_specs.get_activation_tables = patched
    bacc.get_activation_tables = patched


def _bg_chunks(BGP: int):
    """Chunk the bg axis into 1024-wide pieces; fold any remainder into the
    last chunk (so it is 1024..1536 wide -> at most 3 PSUM banks)."""
    n = max(1, BGP // 1024)
    chunks = [(i * 1024, 1024) for i in range(n)]
    rem = BGP - n * 1024
    off, w = chunks[-1]
    chunks[-1] = (off, w + rem)
    assert chunks[-1][1] <= 1536
    return chunks


def _build_nc(FG_TILES: int, BG_TILES: int, CB: int):
    """SPMD single-core program: fg-rows x bg-cols masked softmax-denominator."""
    _install_act_table_patch()
    f32 = mybir.dt.float32
    bf16 = mybir.dt.bfloat16
    FGP = 128 * FG_TILES
    BGP = 128 * BG_TILES

    nc = bacc.Bacc("TRN2", target_bir_lowering=False, debug=False)

    ffg_d = nc.dram_tensor("ffg", [CB, 128, FGP], bf16, kind="ExternalInput")
    fbg_d = nc.dram_tensor("fbg", [CB, 128, BGP], bf16, kind="ExternalInput")
    rm_d = nc.dram_tensor("rowmask", [128, FG_TILES], f32, kind="ExternalInput")
    cst_d = nc.dram_tensor("consts", [128, 2], f32, kind="ExternalInput")
    out_d = nc.dram_tensor("out", [1, 1], f32, kind="ExternalOutput")

    chunks = _bg_chunks(BGP)
    NB = len(chunks)
    LW = chunks[-1][1]  # last (widest) chunk

    with tile.TileContext(nc) as tc:
        with (
            tc.tile_pool(name="persist", bufs=1) as P,
            tc.tile_pool(name="scratch", bufs=3) as S,
            tc.tile_pool(name="sp_psum", bufs=1, space="PSUM") as SP,
            tc.tile_pool(name="mm_psum", bufs=2, space="PSUM") as MP,
            tc.tile_pool(name="ml_psum", bufs=1, space="PSUM") as ML,
        ):
            # ---- input DMA: paired issue on sync+scalar (c0/c1), in dependency
            # order: bg chunk0, fg first half, bg tail, fg second half.  Queue
            # rings are FIFO so first-issued descriptors complete first; the fg
            # second half is only needed from row-tile TA onwards. ----
            TA = (FG_TILES + 1) // 2  # fg row-tiles covered by the first half
            HFa = 128 * TA
            c0w = chunks[0][1]
            fbgb = [P.tile([128, BGP], bf16, name=f"fbgb_{c}") for c in range(CB)]
            ffgb = [P.tile([128, FGP], bf16, name=f"ffgb_{c}") for c in range(CB)]
            eng_pair = [nc.sync, nc.scalar]
            for c in range(CB):
                eng_pair[c % 2].dma_start(fbgb[c][:, 0:c0w], fbg_d[c, :, 0:c0w])
            for c in range(CB):
                eng_pair[c % 2].dma_start(ffgb[c][:, 0:HFa], ffg_d[c, :, 0:HFa])
            for off, w in chunks[1:]:
                for c in range(CB):
                    eng_pair[c % 2].dma_start(
                        fbgb[c][:, off : off + w], fbg_d[c, :, off : off + w]
                    )
            for c in range(CB):
                eng_pair[c % 2].dma_start(ffgb[c][:, HFa:FGP], ffg_d[c, :, HFa:FGP])
            consts = P.tile([128, 2], f32)
            nc.gpsimd.dma_start(consts[:], cst_d[:, :])
            rmask = P.tile([128, FG_TILES], f32)
            nc.gpsimd.dma_start(rmask[:], rm_d[:, :])

            ones_bf = P.tile([128, 128], bf16)
            nc.vector.memset(ones_bf[:], 1.0)
            ones_f = P.tile([128, 1], f32)
            nc.vector.memset(ones_f[:], 1.0)
            bias_tiny = P.tile([128, 1], f32)
            nc.vector.memset(bias_tiny[:], 1e-30)
            bias_lnt = P.tile([128, 1], f32)
            nc.vector.memset(bias_lnt[:], float(math.log(1.0 / TEMP)))

            # ---- PE warmup: ~2.5us of dummy matmuls while DMA lands, so the
            # HAM clock-gate reaches 2.4GHz before the real matmuls start ----
            wu = MP.tile([128, 1024], f32, name="g0", tag="g0")
            for _ in range(24):
                nc.tensor.matmul(wu[:, 0:128], ones_bf[:, :], ones_bf[:, :])

            # ---- bg pipeline, chunk-major: f2 -> ones-matmul -> ln -> exp -> mul
            f2bg = [P.tile([128, BGP], bf16, name=f"f2bg_{c}") for c in range(CB)]
            lnbg = P.tile([128, BGP], f32)
            rnbg = P.tile([128, BGP], bf16)
            fbgn = [P.tile([128, BGP], bf16, name=f"fbgn_{c}") for c in range(CB)]

            def bg_norm_chunk(off, w, tag, sub=1024):
                """sub < w pipelines the chain at finer granularity (chunk0)."""
                for s0 in range(0, w, sub):
                    sw = min(sub, w - s0)
                    for c in range(CB):
                        nc.vector.tensor_mul(
                            f2bg[c][:, off + s0 : off + s0 + sw],
                            fbgb[c][:, off + s0 : off + s0 + sw],
                            fbgb[c][:, off + s0 : off + s0 + sw],
                        )
                if tag == "g0":
                    ps = MP.tile([128, w], f32, name="g0", tag="g0")
                else:
                    ps = ML.tile([128, w], f32, name="gl", tag="gl")
                for s in range(0, w, 512):
                    ws = min(512, w - s)
                    for c in range(CB):
                        nc.tensor.matmul(
                            ps[:, s : s + ws],
                            ones_bf[:, :],
                            f2bg[c][:, off + s : off + s + ws],
                            start=(c == 0),
                            stop=(c == CB - 1),
                        )
                for s0 in range(0, w, sub):
                    sw = min(sub, w - s0)
                    # ln(norm2); +1e-30 keeps padded (all-zero) columns finite
                    nc.scalar.activation(
                        lnbg[:, off + s0 : off + s0 + sw],
                        ps[:, s0 : s0 + sw],
                        AF.Ln,
                        bias=bias_tiny[:, :],
                    )
                    # rnorm = exp(-0.5*ln) = 1/sqrt(norm2); padded cols stay 0
                    nc.scalar.activation(
                        rnbg[:, off + s0 : off + s0 + sw],
                        lnbg[:, off + s0 : off + s0 + sw],
                        AF.Exp,
                        scale=-0.5,
                    )
                    for c in range(CB):
                        nc.vector.tensor_mul(
                            fbgn[c][:, off + s0 : off + s0 + sw],
                            fbgb[c][:, off + s0 : off + s0 + sw],
                            rnbg[:, off + s0 : off + s0 + sw],
                        )

            bg_norm_chunk(*chunks[0], tag="g0", sub=512)

            # ---- fg norms -> per-row ACT scale 1/(norm*TEMP), in two halves so
            # scfg[:, 0:TA] (all the early row-tiles need) is ready early ----
            f2fg = [P.tile([128, FGP], bf16, name=f"f2fg_{c}") for c in range(CB)]
            ps2 = SP.tile([128, FG_TILES], f32, name="sp", tag="sp")
            lnfg = P.tile([128, FG_TILES], f32)
            scfg = P.tile([128, FG_TILES], f32)

            def fg_scale_half(t0, t1):
                j0, j1 = 128 * t0, 128 * t1
                for c in range(CB):
                    eng = nc.gpsimd if c == 0 else nc.vector
                    eng.tensor_mul(
                        f2fg[c][:, j0:j1], ffgb[c][:, j0:j1], ffgb[c][:, j0:j1]
                    )
                for i in range(t0, t1):
                    for c in range(CB):
                        nc.tensor.matmul(
                            ps2[:, i : i + 1],
                            f2fg[c][:, 128 * i : 128 * (i + 1)],
                            ones_bf[:, 0:1],
                            start=(c == 0),
                            stop=(c == CB - 1),
                        )
                nc.scalar.activation(
                    lnfg[:, t0:t1], ps2[:, t0:t1], AF.Ln, bias=bias_tiny[:, :]
                )
                # scale_fg = exp(-0.5*ln(norm2) + ln(1/TEMP)) = 1/(norm*TEMP)
                nc.scalar.activation(
                    scfg[:, t0:t1], lnfg[:, t0:t1], AF.Exp, scale=-0.5, bias=bias_lnt[:, :]
                )

            fg_scale_half(0, TA)
            for off, w in chunks[1:]:
                bg_norm_chunk(off, w, tag="gl")
            fg_scale_half(TA, FG_TILES)

            # ---- main loop: G tiles -> exp with fused row-sum on ACT ----
            # The wide "gl" chunk is single-buffered (PSUM bank budget); that
            # hides fine mid-loop but would serialize PE behind ACT at the very
            # end, so the last row-tile instead uses double-buffered g0 slots
            # plus a small SP-bank remainder.
            last_chunks = []
            off = 0
            while off + 1024 <= BGP:
                last_chunks.append((off, 1024, "g0"))
                off += 1024
            if off < BGP:
                last_chunks.append((off, BGP - off, "sp"))
            NBMAX = max(NB, len(last_chunks))
            negacc = P.tile([128, FG_TILES * NBMAX], f32)
            nc.vector.memset(negacc[:], 0.0)
            for mi in range(FG_TILES):
                if mi == FG_TILES - 1:
                    mychunks = last_chunks
                else:
                    mychunks = [
                        (off, w, "g0" if j < NB - 1 else "gl")
                        for j, (off, w) in enumerate(chunks)
                    ]
                gts = []
                for off, w, tag in mychunks:
                    if tag == "g0":
                        gt = MP.tile([128, w], f32, name="g0", tag="g0")
                    elif tag == "gl":
                        gt = ML.tile([128, w], f32, name="gl", tag="gl")
                    else:
                        gt = SP.tile([128, w], f32, name="sp", tag="sp")
                    gts.append(gt)
                for c in range(CB):
                    lhsT = ffgb[c][:, 128 * mi : 128 * (mi + 1)]
                    for gt, (off, w, tag) in zip(gts, mychunks):
                        for s in range(0, w, 512):
                            ws = min(512, w - s)
                            nc.tensor.matmul(
                                gt[:, s : s + ws],
                                lhsT,
                                fbgn[c][:, off + s : off + s + ws],
                                start=(c == 0),
                                stop=(c == CB - 1),
                            )
                for j, (gt, (off, w, tag)) in enumerate(zip(gts, mychunks)):
                    es = S.tile([128, LW], bf16, name="es", tag=f"es{j}")
                    nc.scalar.activation(
                        es[:, 0:w],
                        gt[:, :],
                        AF.Exp,
                        scale=scfg[:, mi : mi + 1],
                        accum_out=negacc[:, mi * NBMAX + j : mi * NBMAX + j + 1],
                    )

            # ---- epilogue ----
            negsum = P.tile([128, FG_TILES], f32)
            nc.vector.tensor_reduce(
                negsum[:, :],
                negacc[:, :].rearrange("p (m j) -> p m j", j=NBMAX),
                axis=AX.X,
                op=ALU.add,
            )
            plog = P.tile([128, FG_TILES], f32)
            # A = POS + EPS - n_bg_pad folds the padded-column correction into the bias
            nc.scalar.activation(plog[:, :], negsum[:, :], AF.Ln, bias=consts[:, 0:1])
            masked = P.tile([128, FG_TILES], f32)
            nc.vector.scalar_tensor_tensor(
                masked[:, :], plog[:, :], -LOGPOS, rmask[:, :], op0=ALU.add, op1=ALU.mult
            )
            red = P.tile([128, 1], f32)
            nc.vector.tensor_reduce(red[:, :], masked[:, :], axis=AX.X, op=ALU.add)
            ps3 = SP.tile([1, 1], f32, name="sp", tag="sp")
            nc.tensor.matmul(ps3[:, :], red[:, :], ones_f[:, :])
            res = P.tile([1, 1], f32)
            nc.scalar.activation(res[:, :], ps3[:, :], AF.Copy, scale=consts[0:1, 1:2])
            nc.sync.dma_start(out_d[:, :], res[:, :])

    nc.compile()
    return nc


def _get_nc(FG_TILES: int, BG_TILES: int, CB: int):
    key = (FG_TILES, BG_TILES, CB)
    if key not in _NC_CACHE:
        _NC_CACHE[key] = _build_nc(FG_TILES, BG_TILES, CB)
    return _NC_CACHE[key]


def kernel(**inputs) -> np.ndarray:
    cur = np.asarray(inputs["current_preds"], dtype=np.float32)
    hist = np.asarray(inputs["history_preds"], dtype=np.float32)
    feats = np.asarray(inputs["features"], dtype=np.float32)

    T = cur.shape[0]
    C = feats.shape[1]
    N = int(np.prod(cur.shape[1:]))
    CB = C // 128

    # ---- labels (fp32 math mirroring the reference) ----
    cb = (cur > 0.5).astype(np.float32).reshape(T, -1)
    hb = (hist > 0.5).astype(np.float32).reshape(T, -1)
    e1 = (cb * hb).sum(axis=1, dtype=np.float32)
    e2 = cb.sum(axis=1, dtype=np.float32) + hb.sum(axis=1, dtype=np.float32)
    m1 = (np.float32(2.0) * e1 + np.float32(EPS)) / (e2 + np.float32(EPS))
    m2 = (e1 + np.float32(EPS)) / (e2 - e1 + np.float32(EPS))
    dev = np.float32(1.0) - (m1 + m2) / np.float32(2.0)
    use_curr = dev <= np.float32(THRESH)
    labels = np.where(use_curr[:, None, None, None], cur, hist).astype(np.float32)

    lbl = labels.reshape(T, N)
    fg = lbl > 0.5
    nfg = fg.sum(axis=1).astype(np.int64)
    nbg = N - nfg
    valid = (nfg > 0) & (nbg > 0)

    FG_TILES = max(1, int(-(-int(nfg.max()) // 128)))
    BG_TILES = max(1, int(-(-int(nbg.max()) // 128)))
    # last merged bg chunk must stay within 3 PSUM banks (<=1536 wide)
    while 128 * BG_TILES >= 1024 and (128 * BG_TILES) % 1024 > 512:
        BG_TILES += 1
    FGP, BGP = 128 * FG_TILES, 128 * BG_TILES

    in_maps = []
    for t in range(T):
        f = feats[t].reshape(C, N)
        m = fg[t]
        ffg = np.zeros((C, FGP), dtype=ml_dtypes.bfloat16)
        ffg[:, : nfg[t]] = f[:, m].astype(ml_dtypes.bfloat16)
        fbg = np.zeros((C, BGP), dtype=ml_dtypes.bfloat16)
        fbg[:, : nbg[t]] = f[:, ~m].astype(ml_dtypes.bfloat16)
        rowmask = (
            (np.arange(FGP).reshape(FG_TILES, 128).T < nfg[t]).astype(np.float32)
        )
        A = np.float32(POS + EPS - float(BGP - nbg[t]))
        inv_nfg = np.float32(1.0 / max(float(nfg[t]), 1.0))
        consts = np.tile(np.array([[A, inv_nfg]], dtype=np.float32), (128, 1))
        in_maps.append(
            {
                "ffg": np.ascontiguousarray(ffg.reshape(CB, 128, FGP)),
                "fbg": np.ascontiguousarray(fbg.reshape(CB, 128, BGP)),
                "rowmask": np.ascontiguousarray(rowmask),
                "consts": consts,
            }
        )

    nc = _get_nc(FG_TILES, BG_TILES, CB)
    res = run_bass_kernel_spmd(nc, in_maps, core_ids=list(range(T)))
    global LAST_RESULTS
    LAST_RESULTS = res

    fls = np.array([res.results[t]["out"][0, 0] for t in range(T)], dtype=np.float32)
    n_valid = int(valid.sum())
    if n_valid > 0:
        loss = np.float32((fls * valid.astype(np.float32)).sum() / max(n_valid, 1))
    else:
        loss = np.float32(0.0)
    return labels, np.asarray(loss, dtype=np.float32)
